# revision 6
# baseline (speedup 1.0000x reference)
"""
Trainium2 Bass kernel for nn_CurvePredictor: encoder MLP -> 2-layer LSTM
(T=256 steps, layer-0 input constant) -> decoder MLP.  B=2048, H=256.

Sharding: pure data parallel over the batch dim across 8 cores (B_local=256);
all weights replicated.

Device layout is "feature-major": hidden/gate dims live on SBUF partitions,
batch lives on the free dim, so the recurrent matmuls need no runtime
transposes (h state tiles are directly the next step's moving operand).

Gate rows are permuted host-side to [i, f, o, g] so one fused sigmoid
activation covers i,f (and a second covers o), with tanh on g; gates PSUM is
written as two [128, 1024] halves (m-slices x batch).

All weight tensors are pre-permuted/pre-transposed/pre-reshaped on the host
into the exact SBUF layouts the matmuls want, so the device only does
contiguous DMAs.

gelu(x) (exact, erf-based) is computed as (0.5 + 0.5*erf(x/sqrt(2))) * x so
the whole kernel stays inside one ACT table set (sigmoid/tanh/erf); the
decoder's 0.5 is folded into W4.
"""

import numpy as np
from contextlib import ExitStack

import concourse.bacc as bacc
import concourse.bass as bass
import concourse.mybir as mybir
import concourse.tile as tile
from concourse import bass_utils

F32 = mybir.dt.float32
AF = mybir.ActivationFunctionType
ALU = mybir.AluOpType

B, T, H = 2048, 256, 256
N_CORES = 8
BL = B // N_CORES  # 256 batch rows per core
G4 = 4 * H         # 1024 gate rows
EPS = 1e-5
RS2 = float(1.0 / np.sqrt(2.0))  # 1/sqrt(2)

# permuted gate-block order: i, f, o, g  (torch order in weights is i,f,g,o)
_PERM = np.concatenate([
    np.arange(0, 256),      # i
    np.arange(256, 512),    # f
    np.arange(768, 1024),   # o
    np.arange(512, 768),    # g
]).astype(np.int64)


def _to_km(wT):
    """[K, M] -> SBUF layout [128, (K/128)*M]: k-tile-major along free dim."""
    K, M = wT.shape
    nk = K // 128
    return np.ascontiguousarray(
        wT.reshape(nk, 128, M).transpose(1, 0, 2).reshape(128, nk * M)
    ).astype(np.float32)


def prep_host_inputs(x, W1, b1, g1, be1, W2, b2, g2, be2,
                     Wih0, Whh0, bih0, bhh0, Wih1, Whh1, bih1, bhh1,
                     W3, b3, W4, b4):
    """Build the per-core in_maps (numpy only)."""
    f32 = lambda a: np.asarray(a, dtype=np.float32)
    x = f32(x)
    Wih0p = f32(Wih0)[_PERM]          # [1024, 256]
    Whh0p = f32(Whh0)[_PERM]          # [1024, 256]
    Wc1p = np.concatenate([f32(Wih1), f32(Whh1)], axis=1)[_PERM]  # [1024, 512]
    bb0 = (f32(bih0) + f32(bhh0))[_PERM]  # [1024]
    bb1 = (f32(bih1) + f32(bhh1))[_PERM]  # [1024]

    common = {
        "w1T": np.ascontiguousarray(f32(W1).T),            # [5, 128]
        "w2T": _to_km(f32(W2).T),                          # [128, 256]
        "wih0T": _to_km(Wih0p.T),                          # [128, 2048]
        "whh0T": _to_km(Whh0p.T),                          # [128, 2048]
        "wc1T": _to_km(Wc1p.T),                            # [128, 4096]
        "w3T": _to_km(f32(W3).T),                          # [128, 256]
        "w4T": np.ascontiguousarray(0.5 * f32(W4).T),      # [128, 2]
        "ident": np.eye(128, dtype=np.float32),
        "b1c": f32(b1).reshape(128, 1),
        "b1sc": (f32(b1) * RS2).reshape(128, 1),
        "g1c": f32(g1).reshape(128, 1),
        "be1c": f32(be1).reshape(128, 1),
        "b2c": f32(b2).reshape(256, 1),
        "b2sc": (f32(b2) * RS2).reshape(256, 1),
        "g2c": f32(g2).reshape(256, 1),
        "be2c": f32(be2).reshape(256, 1),
        "bb0r": bb0.reshape(1, 1024),
        "bb1r": bb1.reshape(1, 1024),
        "b3c": f32(b3).reshape(128, 1),
        "b3sc": (f32(b3) * RS2).reshape(128, 1),
        "b4c": f32(b4).reshape(2, 1),
    }
    flags = {
        "b1": bool(np.any(common["b1c"])),
        "b2": bool(np.any(common["b2c"])),
        "ln1_aff": bool(np.any(common["be1c"])) or not np.all(common["g1c"] == 1.0),
        "ln2_aff": bool(np.any(common["be2c"])) or not np.all(common["g2c"] == 1.0),
        "bb0": bool(np.any(bb0)),
        "bb1": bool(np.any(bb1)),
        "b3": bool(np.any(common["b3c"])),
        "b4": bool(np.any(common["b4c"])),
    }
    in_maps = []
    for c in range(N_CORES):
        m = dict(common)
        m["xT"] = np.ascontiguousarray(x[c * BL:(c + 1) * BL].T)  # [5, BL]
        in_maps.append(m)
    return in_maps, flags


def _gelu_from_psum(nc, pools, psz, bias_c, bias_sc, has_b, name):
    """u = gelu(psz + bias) * 2ish: returns SBUF tile u = (z+b)*(1+erf((z+b)/sqrt2)) * 0.5
    i.e. exact gelu. psz is a PSUM tile [P, N]."""
    P, N = psz.shape
    sb = pools["sb_tmp"]
    e = sb.tile([P, N], F32, tag=f"{name}_e", name=f"{name}_e")
    # e = erf(z*rs2 + b*rs2)
    nc.scalar.activation(e[:], psz[:], AF.Erf, bias=bias_sc if has_b else 0.0,
                         scale=RS2)
    # e2 = 0.5*e + 0.5
    e2 = sb.tile([P, N], F32, tag=f"{name}_e2", name=f"{name}_e2")
    nc.vector.tensor_scalar(e2[:], e[:], 0.5, 0.5, ALU.mult, ALU.add)
    u = sb.tile([P, N], F32, tag=f"{name}_u", name=f"{name}_u")
    if has_b:
        zb = sb.tile([P, N], F32, tag=f"{name}_zb", name=f"{name}_zb")
        nc.scalar.activation(zb[:], psz[:], AF.Identity, bias=bias_c)
        nc.vector.tensor_mul(u[:], e2[:], zb[:])
    else:
        nc.vector.tensor_mul(u[:], e2[:], psz[:])
    return u


def _ln_feature_major(nc, pools, u_tiles, ones_c, ones_k1, g_c, be_c, affine,
                      name):
    """LayerNorm across the partition (feature) axis for a list of [128, N]
    SBUF tiles forming the feature dim.  Returns list of normalized tiles."""
    sb = pools["sb_tmp"]
    ps_small = pools["psum_small"]
    nfeat = 128 * len(u_tiles)
    N = u_tiles[0].shape[1]

    pssum = ps_small.tile([1, N], F32, tag="zw", name=f"{name}_pssum")
    for i, u in enumerate(u_tiles):
        nc.tensor.matmul(pssum[:], ones_c[:], u[:], start=(i == 0),
                         stop=(i == len(u_tiles) - 1))
    m = sb.tile([1, N], F32, tag=f"{name}_m", name=f"{name}_m")
    nc.vector.tensor_scalar_mul(m[:], pssum[:], 1.0 / nfeat)

    pssq = ps_small.tile([1, N], F32, tag="zw", name=f"{name}_pssq")
    for i, u in enumerate(u_tiles):
        sq = sb.tile([128, N], F32, tag=f"{name}_sq", name=f"{name}_sq")
        nc.scalar.activation(sq[:], u[:], AF.Square)
        nc.tensor.matmul(pssq[:], ones_c[:], sq[:], start=(i == 0),
                         stop=(i == len(u_tiles) - 1))
    msq = sb.tile([1, N], F32, tag=f"{name}_msq", name=f"{name}_msq")
    nc.vector.tensor_mul(msq[:], m[:], m[:])
    # vpe = sumsq/nfeat - m^2 + eps
    vpe = sb.tile([1, N], F32, tag=f"{name}_vpe", name=f"{name}_vpe")
    nc.vector.scalar_tensor_tensor(vpe[:], pssq[:], 1.0 / nfeat, msq[:],
                                   ALU.mult, ALU.subtract)
    nc.vector.tensor_scalar_add(vpe[:], vpe[:], EPS)
    rec = sb.tile([1, N], F32, tag=f"{name}_rec", name=f"{name}_rec")
    nc.vector.reciprocal(rec[:], vpe[:])
    r0 = sb.tile([1, N], F32, tag=f"{name}_r0", name=f"{name}_r0")
    nc.scalar.activation(r0[:], rec[:], AF.Sqrt)
    # one Newton step (ACT sqrt is low precision): negr = (0.5*vpe*r0^2 - 1.5)*r0 = -rsqrt
    t1 = sb.tile([1, N], F32, tag=f"{name}_t1", name=f"{name}_t1")
    nc.vector.tensor_mul(t1[:], r0[:], r0[:])
    t2 = sb.tile([1, N], F32, tag=f"{name}_t2", name=f"{name}_t2")
    nc.vector.scalar_tensor_tensor(t2[:], vpe[:], 0.5, t1[:], ALU.mult, ALU.mult)
    negr = sb.tile([1, N], F32, tag=f"{name}_negr", name=f"{name}_negr")
    nc.vector.scalar_tensor_tensor(negr[:], t2[:], 1.5, r0[:], ALU.subtract,
                                   ALU.mult)
    # broadcast m and negr across partitions via K=1 matmuls
    psm = ps_small.tile([128, N], F32, tag="zw", name=f"{name}_psm")
    nc.tensor.matmul(psm[:], ones_k1[:], m[:], start=True, stop=True)
    psr = ps_small.tile([128, N], F32, tag="zw", name=f"{name}_psr")
    nc.tensor.matmul(psr[:], ones_k1[:], negr[:], start=True, stop=True)

    outs = []
    for i, u in enumerate(u_tiles):
        # t = m - u  (note order: combined with negr this gives (u-m)*rsqrt)
        t = sb.tile([128, N], F32, tag=f"{name}_t", name=f"{name}_t")
        nc.vector.tensor_sub(t[:], psm[:], u[:])
        y = sb.tile([128, N], F32, tag=f"{name}_y{i}", name=f"{name}_y{i}")
        nc.vector.tensor_mul(y[:], t[:], psr[:])          # (m-u)*(-r) = (u-m)*r
        if affine:
            ga = g_c[:, i:i + 1]
            bea = be_c[:, i:i + 1]
            yf = sb.tile([128, N], F32, tag=f"{name}_yf{i}", name=f"{name}_yf{i}")
            nc.scalar.activation(yf[:], y[:], AF.Identity, bias=bea, scale=ga)
            outs.append(yf)
        else:
            outs.append(y)
    return outs


def build_program(t_steps, flags, b_local=BL):
    """Builds and compiles the Bacc program. Returns nc."""
    nc = bacc.Bacc("TRN2", target_bir_lowering=False, debug=False,
                   enable_asserts=False, num_devices=1)

    NB = b_local

    # ---- DRAM I/O ----
    d = {}
    def din(name, shape):
        d[name] = nc.dram_tensor(name, shape, F32, kind="ExternalInput").ap()
    din("xT", [5, NB])
    din("w1T", [5, 128]); din("w2T", [128, 256])
    din("wih0T", [128, 2048]); din("whh0T", [128, 2048]); din("wc1T", [128, 4096])
    din("w3T", [128, 256]); din("w4T", [128, 2]); din("ident", [128, 128])
    din("b1c", [128, 1]); din("b1sc", [128, 1]); din("g1c", [128, 1]); din("be1c", [128, 1])
    din("b2c", [256, 1]); din("b2sc", [256, 1]); din("g2c", [256, 1]); din("be2c", [256, 1])
    din("bb0r", [1, 1024]); din("bb1r", [1, 1024])
    din("b3c", [128, 1]); din("b3sc", [128, 1]); din("b4c", [2, 1])
    outT = nc.dram_tensor("outT", [t_steps, 2, NB], F32, kind="ExternalOutput").ap()

    with tile.TileContext(nc) as tc, ExitStack() as ctx:
        wpool = ctx.enter_context(tc.tile_pool(name="weights", bufs=1))
        sb_tmp = ctx.enter_context(tc.tile_pool(name="sb_tmp", bufs=2))
        psum_g = ctx.enter_context(
            tc.tile_pool(name="psum_g", bufs=3, space="PSUM"))
        psum_small = ctx.enter_context(
            tc.tile_pool(name="psum_small", bufs=2, space="PSUM"))
        pools = {"sb_tmp": sb_tmp, "psum_small": psum_small, "psum_g": psum_g}

        # ---- load weights into SBUF ----
        def wload(name, shape):
            tl = wpool.tile(shape, F32, name=f"sb_{name}")
            nc.sync.dma_start(tl[:], d[name][:])
            return tl
        w1T = wload("w1T", [5, 128]); w2T = wload("w2T", [128, 256])
        wih0T = wload("wih0T", [128, 2048]); whh0T = wload("whh0T", [128, 2048])
        wc1T = wload("wc1T", [128, 4096])
        w3T = wload("w3T", [128, 256]); w4T = wload("w4T", [128, 2])
        ident = wload("ident", [128, 128])
        xT = wload("xT", [5, NB])
        b1c = wload("b1c", [128, 1]); b1sc = wload("b1sc", [128, 1])
        g1c = wload("g1c", [128, 1]); be1c = wload("be1c", [128, 1])
        def wload2(name):
            # [256, 1] DRAM vector -> one [128, 2] SBUF tile; column i holds
            # feature rows [i*128:(i+1)*128].
            tl = wpool.tile([128, 2], F32, name=f"sb_{name}")
            for i in range(2):
                nc.sync.dma_start(tl[:, i:i + 1], d[name][i * 128:(i + 1) * 128, :])
            return tl
        b2c = wload2("b2c"); b2sc = wload2("b2sc")
        g2c = wload2("g2c"); be2c = wload2("be2c")
        bb0r = wload("bb0r", [1, 1024]); bb1r = wload("bb1r", [1, 1024])
        b3c = wload("b3c", [128, 1]); b3sc = wload("b3sc", [128, 1])
        b4c = wload("b4c", [2, 1])

        ones_c = wpool.tile([128, 1], F32, name="ones_c")
        nc.vector.memset(ones_c[:], 1.0)
        ones_k1 = wpool.tile([1, 128], F32, name="ones_k1")
        nc.vector.memset(ones_k1[:], 1.0)
        ones_nb = wpool.tile([1, NB], F32, name="ones_nb")
        nc.vector.memset(ones_nb[:], 1.0)

        # ================= encoder =================
        # L1: z1.T = W1 @ x.T  -> [128, NB]
        psz1 = psum_small.tile([128, NB], F32, tag="zw", name="psz1")
        nc.tensor.matmul(psz1[:], w1T[:], xT[:], start=True, stop=True)
        u1 = _gelu_from_psum(nc, pools, psz1, b1c, b1sc, flags["b1"], "g1")
        y1 = _ln_feature_major(nc, pools, [u1], ones_c, ones_k1, g1c, be1c,
                               flags["ln1_aff"], "ln1")[0]
        # L2: z2.T = W2 @ y1 -> two m-tiles [128, NB]
        u2s = []
        for mi in range(2):
            psz2 = psum_small.tile([128, NB], F32, tag="zw", name=f"psz2_{mi}")
            nc.tensor.matmul(psz2[:], w2T[:, mi * 128:(mi + 1) * 128], y1[:],
                             start=True, stop=True)
            u2s.append(_gelu_from_psum(
                nc, pools, psz2, b2c[:, mi:mi + 1],
                b2sc[:, mi:mi + 1], flags["b2"], f"g2_{mi}"))
        encT = _ln_feature_major(nc, pools, u2s, ones_c, ones_k1, g2c, be2c,
                                 flags["ln2_aff"], "ln2")
        # encT: 2 tiles [128, NB] = k-tiles of enc.T

        # ================= xp0 = Wih0p @ enc.T (+bb0) ==================
        xp0 = wpool.tile([128, 2048], F32, name="xp0")
        for half in range(2):
            ps = psum_g.tile([128, 1024], F32, tag="g", name=f"xp0ps_{half}")
            for mloc in range(4):
                mm = half * 4 + mloc
                for k in range(2):
                    nc.tensor.matmul(
                        ps[:, mloc * NB:(mloc + 1) * NB],
                        wih0T[:, k * 1024 + mm * 128: k * 1024 + (mm + 1) * 128],
                        encT[k][:],
                        start=(k == 0), stop=(k == 1 and not flags["bb0"]))
            if flags["bb0"]:
                for mloc in range(4):
                    mm = half * 4 + mloc
                    nc.tensor.matmul(
                        ps[:, mloc * NB:(mloc + 1) * NB],
                        bb0r[:, mm * 128:(mm + 1) * 128],
                        ones_nb[:], start=False, stop=True)
            nc.vector.tensor_copy(xp0[:, half * 1024:(half + 1) * 1024], ps[:])

        # bias1 broadcast tile (only if nonzero)
        if flags["bb1"]:
            bb1bc = wpool.tile([128, 2048], F32, name="bb1bc")
            for half in range(2):
                ps = psum_g.tile([128, 1024], F32, tag="g", name=f"bb1ps_{half}")
                for mloc in range(4):
                    mm = half * 4 + mloc
                    nc.tensor.matmul(
                        ps[:, mloc * NB:(mloc + 1) * NB],
                        bb1r[:, mm * 128:(mm + 1) * 128],
                        ones_nb[:], start=True, stop=True)
                nc.vector.tensor_copy(bb1bc[:, half * 1024:(half + 1) * 1024],
                                      ps[:])

        # ================= LSTM scan ==================
        spool = ctx.enter_context(tc.tile_pool(name="spool", bufs=3))
        gpool = ctx.enter_context(tc.tile_pool(name="gpool", bufs=2))
        cpool = ctx.enter_context(tc.tile_pool(name="cpool", bufs=2))
        tpool = ctx.enter_context(tc.tile_pool(name="tpool", bufs=2))
        opool = ctx.enter_context(tc.tile_pool(name="opool", bufs=2))

        S_prev = spool.tile([128, 1024], F32, tag="S", name="S_init0")
        nc.vector.memset(S_prev[:], 0.0)
        S_cur = spool.tile([128, 1024], F32, tag="S", name="S_init1")
        nc.vector.memset(S_cur[:], 0.0)
        c_prev = [None, None]
        for l in range(2):
            c_prev[l] = cpool.tile([128, 512], F32, tag=f"c{l}", name=f"c{l}_init")
            nc.vector.memset(c_prev[l][:], 0.0)

        for t in range(t_steps):
            S_next = spool.tile([128, 1024], F32, tag="S", name=f"S_{t + 1}")

            # ---------- layer 0 ----------
            G0 = gpool.tile([128, 2048], F32, tag="G0", name=f"G0_{t}")
            psH = [None, None]
            for half in range(2):
                ps = psum_g.tile([128, 1024], F32, tag="g", name=f"ps0_{t}_{half}")
                psH[half] = ps
                # inject xp0 (includes bb0)
                for j in range(2):
                    nc.tensor.matmul(
                        ps[:, j * 512:(j + 1) * 512], ident[:],
                        xp0[:, half * 1024 + j * 512: half * 1024 + (j + 1) * 512],
                        start=True, stop=False, skip_group_check=True)
                for mloc in range(4):
                    mm = half * 4 + mloc
                    for k in range(2):
                        nc.tensor.matmul(
                            ps[:, mloc * NB:(mloc + 1) * NB],
                            whh0T[:, k * 1024 + mm * 128: k * 1024 + (mm + 1) * 128],
                            S_prev[:, k * NB:(k + 1) * NB],
                            start=False, stop=(k == 1), skip_group_check=True)
            # activations: half A = i,f (sigmoid); half B = o (sigmoid), g (tanh)
            nc.scalar.activation(G0[:, 0:1024], psH[0][:], AF.Sigmoid)
            nc.scalar.activation(G0[:, 1024:1536], psH[1][:, 0:512], AF.Sigmoid)
            nc.scalar.activation(G0[:, 1536:2048], psH[1][:, 512:1024], AF.Tanh)
            # c0 = f*c0 + i*g
            p1 = tpool.tile([128, 512], F32, tag="p1_0", name=f"p1_0_{t}")
            nc.vector.tensor_mul(p1[:], G0[:, 0:512], G0[:, 1536:2048])
            fc = tpool.tile([128, 512], F32, tag="fc_0", name=f"fc_0_{t}")
            nc.gpsimd.tensor_mul(fc[:], G0[:, 512:1024], c_prev[0][:])
            c0 = cpool.tile([128, 512], F32, tag="c0", name=f"c0_{t}")
            nc.vector.tensor_add(c0[:], p1[:], fc[:])
            tnh0 = tpool.tile([128, 512], F32, tag="tnh_0", name=f"tnh_0_{t}")
            nc.scalar.activation(tnh0[:], c0[:], AF.Tanh)
            # h0 -> S_cur[:, 0:512]
            nc.gpsimd.tensor_mul(S_cur[:, 0:512], G0[:, 1024:1536], tnh0[:])
            c_prev[0] = c0

            # ---------- layer 1 ----------
            G1 = gpool.tile([128, 2048], F32, tag="G1", name=f"G1_{t}")
            psH1 = [None, None]
            for half in range(2):
                ps = psum_g.tile([128, 1024], F32, tag="g", name=f"ps1_{t}_{half}")
                psH1[half] = ps
                if flags["bb1"]:
                    for j in range(2):
                        nc.tensor.matmul(
                            ps[:, j * 512:(j + 1) * 512], ident[:],
                            bb1bc[:, half * 1024 + j * 512: half * 1024 + (j + 1) * 512],
                            start=True, stop=False, skip_group_check=True)
                for mloc in range(4):
                    mm = half * 4 + mloc
                    for k in range(4):
                        nc.tensor.matmul(
                            ps[:, mloc * NB:(mloc + 1) * NB],
                            wc1T[:, k * 1024 + mm * 128: k * 1024 + (mm + 1) * 128],
                            S_cur[:, k * NB:(k + 1) * NB],
                            start=(k == 0 and not flags["bb1"]),
                            stop=(k == 3), skip_group_check=True)
            nc.scalar.activation(G1[:, 0:1024], psH1[0][:], AF.Sigmoid)
            nc.scalar.activation(G1[:, 1024:1536], psH1[1][:, 0:512], AF.Sigmoid)
            nc.scalar.activation(G1[:, 1536:2048], psH1[1][:, 512:1024], AF.Tanh)
            p1b = tpool.tile([128, 512], F32, tag="p1_1", name=f"p1_1_{t}")
            nc.vector.tensor_mul(p1b[:], G1[:, 0:512], G1[:, 1536:2048])
            fcb = tpool.tile([128, 512], F32, tag="fc_1", name=f"fc_1_{t}")
            nc.gpsimd.tensor_mul(fcb[:], G1[:, 512:1024], c_prev[1][:])
            c1 = cpool.tile([128, 512], F32, tag="c1", name=f"c1_{t}")
            nc.vector.tensor_add(c1[:], p1b[:], fcb[:])
            tnh1 = tpool.tile([128, 512], F32, tag="tnh_1", name=f"tnh_1_{t}")
            nc.scalar.activation(tnh1[:], c1[:], AF.Tanh)
            # h1 -> S_next[:, 512:1024]
            nc.vector.tensor_mul(S_next[:, 512:1024], G1[:, 1024:1536], tnh1[:])
            c_prev[1] = c1

            # ---------- decoder (fused) ----------
            psz = psum_small.tile([128, NB], F32, tag="zw", name=f"psz_{t}")
            for k in range(2):
                nc.tensor.matmul(
                    psz[:], w3T[:, k * 128:(k + 1) * 128],
                    S_next[:, 512 + k * NB: 512 + (k + 1) * NB],
                    start=(k == 0), stop=(k == 1))
            e = opool.tile([128, NB], F32, tag="e", name=f"e_{t}")
            nc.scalar.activation(e[:], psz[:], AF.Erf,
                                 bias=b3sc[:] if flags["b3"] else 0.0, scale=RS2)
            u = opool.tile([128, NB], F32, tag="u", name=f"u_{t}")
            if flags["b3"]:
                zb = opool.tile([128, NB], F32, tag="zb", name=f"zb_{t}")
                nc.scalar.activation(zb[:], psz[:], AF.Identity, bias=b3c[:])
                nc.vector.scalar_tensor_tensor(u[:], e[:], 1.0, zb[:],
                                               ALU.add, ALU.mult)
            else:
                nc.vector.scalar_tensor_tensor(u[:], e[:], 1.0, psz[:],
                                               ALU.add, ALU.mult)
            psw4 = psum_small.tile([2, NB], F32, tag="zw", name=f"psw4_{t}")
            nc.tensor.matmul(psw4[:], w4T[:], u[:], start=True, stop=True)
            osb = opool.tile([2, NB], F32, tag="osb", name=f"osb_{t}")
            if flags["b4"]:
                nc.vector.scalar_tensor_tensor(osb[:], psw4[:], b4c[:], u[0:2, :],
                                               ALU.add, ALU.bypass)
            else:
                nc.vector.tensor_copy(osb[:], psw4[:])
            nc.sync.dma_start(outT[t], osb[:])

            S_prev, S_cur = S_cur, S_next

    nc.compile()
    return nc


def run_cores(inputs, t_steps=T, n_cores=N_CORES, trace=False):
    """Builds, compiles and runs on hardware. Returns (out [B,T,2], results)."""
    in_maps, flags = prep_host_inputs(**inputs)
    nc = build_program(t_steps, flags)
    res = bass_utils.run_bass_kernel_spmd(
        nc, in_maps[:n_cores], core_ids=list(range(n_cores)), trace=trace)
    outs = []
    for c in range(n_cores):
        o = res.results[c]["outT"]  # [t_steps, 2, BL]
        outs.append(np.ascontiguousarray(np.transpose(o, (2, 0, 1))))
    return np.concatenate(outs, axis=0), res


def kernel(**inputs):
    out, _ = run_cores(inputs)
    return out.astype(np.float32)
